# revision 28
# baseline (speedup 1.0000x reference)
"""Multi-head self-attention Trainium2 Bass kernel (B=2, T=4096, D=512, H=8).

Sharding: 8 cores, each handles (batch b = core//4, head-pair hp = core%4).
Per core, for its 2 heads (host pre-transposes x and pre-scales Wq by 1/8,
all in bf16; the bk bias is dropped entirely — a per-q-column score offset
cancels exactly in softmax):
    kT = Wk @ x.T            ([128, T] bf16: head h on partitions 64h..+63)
    v  = x @ Wv.T            (bf16, interleaved with ones columns)
    qT = Wq' @ x.T + bq'     (q/k/v projections interleaved per x chunk)
    flash attention without max-subtraction (scores ~N(0,1), f32 exp safe):
      per 512-wide q block, per 128-wide kv tile:
        S.T_h0 / S.T_h1 computed as a CONCURRENT row-tiled matmul pair
            (each head contracts only 64 dims -> array rows 0-63 / 64-127)
            into one [128, 1024] PSUM tile (bank per head)
        P.T = exp(S.T) -> bf16. The exp work is SPLIT across two engines:
            3 of 4 kv tiles on ScalarE (LUT exp, one ACTIVATE covering both
            heads); every fourth tile on VectorE via the Schraudolph trick
            (y = s*128/ln2 + (127-.0597)*128 as int16 -> bf16 bit pattern,
            max rel err ~4% which largely cancels between P and l). The
            split both offloads ScalarE and smooths the pipeline (all-ACT
            measured 50us slower). PV consumes P one kv tile behind, so
            the in-order PE queue never stalls on the current tile's exp.
        ctxT_h[+l_h] += vaug_h.T @ P.T_h      ([66, 512] PSUM accumulators,
                                               rows 0..63 ctx.T, 64..65 = l)
    output projection per head as a concurrent row-tiled pair:
        ps_h = ctxT_h.T @ Wo_h  ([128 q, 512]),
    then out = ps_h0 * (1/l_h0) + ps_h1 * (1/l_h1) via per-partition DVE
    scalar ops (1/l transposed to q-on-partitions via a tiny DRAM bounce);
    these output chunks run one q-block behind, inside the exp shadow.
Host gathers: out[b] = sum of 4 cores' partials + (bv @ Wo.T + bo); the
v/o biases fold out exactly because softmax rows sum to 1.

This walrus build accepts at most ONE sync wait per instruction;
split_excess_waits() moves extras onto no-ops. walrus's LDW-dedup pass
(--enable-ldw-opt) rejects row-tiled LDWEIGHTS, so it stays disabled.
"""

import numpy as np
import ml_dtypes

import concourse.bass as bass
import concourse.tile as tile
from concourse import mybir
from concourse.bass_utils import run_bass_kernel_spmd

F32 = mybir.dt.float32
BF16 = mybir.dt.bfloat16
I16 = mybir.dt.int16

N_CORES = 8
B, T, D, H = 2, 4096, 512, 8
DK = D // H          # 64
TT = T // 128        # 32 kv tiles
KC = D // 128        # 4 contraction chunks
QB = 512             # q block width
NQB = T // QB        # 8 q blocks
VW = 132             # vaug cols per kv tile: [v_h0(64) one one v_h1(64) one one]

# Schraudolph exp in bf16 bit domain: bf16(bits(round(s*EXPA + EXPB))) ~ e^s
EXPA = 128.0 / float(np.log(2.0))
EXPB = (127.0 - 0.0597) * 128.0
DVE_EXP_MOD = 4           # kv tiles with kb % MOD == 2 take the DVE exp path

_split_ctr = [0]


def split_excess_waits(nc, limit=1):
    """walrus codegen in this toolchain accepts at most `limit` sync waits
    per instruction; move the excess onto nofuse NoOps inserted right before
    on the same engine (engines execute in order, semantics unchanged)."""
    n_split = 0
    for fn in nc.m.functions:
        blocks = fn.blocks if isinstance(fn.blocks, list) else list(fn.blocks.values())
        for blk in blocks:
            out = []
            for inst in blk.instructions:
                si = inst.sync_info
                if si is not None and len(si.on_wait) > limit:
                    waits = list(si.on_wait)
                    excess, keep = waits[:-limit], waits[-limit:]
                    for w in excess:
                        _split_ctr[0] += 1
                        out.append(mybir.InstNoOp(
                            name=f"I-wsplit-{_split_ctr[0]}",
                            opcode="NoOp",
                            engine=inst.engine,
                            sync_info=mybir.SyncInfo(on_wait=[w], on_update=[]),
                            bass_nofuse=True,
                        ))
                        n_split += 1
                    inst.sync_info = mybir.SyncInfo(
                        on_wait=keep, on_update=list(si.on_update))
                out.append(inst)
            blk.instructions[:] = out
    return n_split


def _ap(src, pattern):
    """Raw AP view over the same tensor/offset with an explicit
    [[stride, size], ...] element-stride pattern (partition dim first)."""
    return bass.AP(tensor=src.tensor, offset=src.offset,
                   ap=[list(d) for d in pattern])


def build_kernel():
    nc = bass.Bass()
    xbT = nc.dram_tensor("xbT", [D, T], BF16, kind="ExternalInput")
    wqT = nc.dram_tensor("wqT", [D, 128], BF16, kind="ExternalInput")
    wkT = nc.dram_tensor("wkT", [D, 128], BF16, kind="ExternalInput")
    wvT = nc.dram_tensor("wvT", [D, 128], BF16, kind="ExternalInput")
    woT = nc.dram_tensor("woT", [128, D], F32, kind="ExternalInput")
    bq = nc.dram_tensor("bq", [128, 1], F32, kind="ExternalInput")
    part = nc.dram_tensor("part", [T, D], F32, kind="ExternalOutput")

    with tile.TileContext(nc) as tc:
        with tc.tile_pool(name="persist", bufs=1) as persist:
            # ---- persistent SBUF (input x streamed on sync+vector queues,
            #      weights on the gpsimd queue so they don't block x) ----
            xTall = persist.tile([128, KC * T], BF16)  # chunk c at cols [c*T,..)
            for n in range(T // 512):
                for c in range(KC):
                    eng = nc.sync if (c % 2 == 0) else nc.scalar
                    eng.dma_start(
                        out=xTall[:, c * T + 512 * n: c * T + 512 * (n + 1)],
                        in_=xbT[128 * c: 128 * (c + 1), 512 * n: 512 * (n + 1)])

            wkt = persist.tile([128, KC, 128], BF16)
            nc.gpsimd.dma_start(out=wkt, in_=wkT.rearrange("(c p) m -> p c m", p=128))
            wvt = persist.tile([128, KC, 128], BF16)
            nc.gpsimd.dma_start(out=wvt, in_=wvT.rearrange("(c p) m -> p c m", p=128))
            wqt = persist.tile([128, KC, 128], BF16)
            nc.gpsimd.dma_start(out=wqt, in_=wqT.rearrange("(c p) m -> p c m", p=128))
            bq_t = persist.tile([128, 1], F32)
            nc.gpsimd.dma_start(out=bq_t, in_=bq[:, :])
            woTf = persist.tile([128, D], F32)
            nc.gpsimd.dma_start(out=woTf, in_=woT[:, :])
            woTs = persist.tile([128, D], BF16)
            nc.vector.tensor_copy(out=woTs, in_=woTf)  # noqa: cast to bf16
            ones2 = persist.tile([128, 2], BF16)
            nc.vector.memset(ones2, 1.0)

            qT2 = persist.tile([128, T], BF16)   # heads stacked [h0|h1]
            kT2 = persist.tile([128, T], BF16)
            vaug = persist.tile([128, TT * VW], BF16)

            # ---- stage B: k and v projections, interleaved per x chunk ----
            with tc.tile_pool(name="psB", bufs=2, space="PSUM") as psB, \
                 tc.tile_pool(name="psV", bufs=2, space="PSUM") as psV:
                for n in range(T // 512):
                    sl = slice(512 * n, 512 * (n + 1))
                    ps_q = psB.tile([128, 512], F32, tag="psq")
                    for c in range(KC):
                        nc.tensor.matmul(
                            ps_q, wqt[:, c, :],
                            xTall[:, c * T + 512 * n: c * T + 512 * (n + 1)],
                            start=(c == 0), stop=(c == KC - 1))
                    nc.vector.tensor_scalar_add(
                        out=qT2[:, sl], in0=ps_q, scalar1=bq_t)
                    ps_k = psB.tile([128, 512], F32, tag="psk")
                    for c in range(KC):
                        nc.tensor.matmul(
                            ps_k, wkt[:, c, :],
                            xTall[:, c * T + 512 * n: c * T + 512 * (n + 1)],
                            start=(c == 0), stop=(c == KC - 1))
                    nc.vector.tensor_copy(out=kT2[:, sl], in_=ps_k)
                    for i in range(4 * n, 4 * n + 4):
                        ps_v = psV.tile([128, 128], F32, tag="psv")
                        for c in range(KC):
                            nc.tensor.matmul(
                                ps_v,
                                xTall[:, c * T + 128 * i: c * T + 128 * (i + 1)],
                                wvt[:, c, :],
                                start=(c == 0), stop=(c == KC - 1))
                        nc.vector.tensor_copy(
                            out=vaug[:, VW * i: VW * i + 64], in_=ps_v[:, 0:64])
                        nc.vector.tensor_copy(
                            out=vaug[:, VW * i + 66: VW * i + 130],
                            in_=ps_v[:, 64:128])
                        nc.vector.tensor_copy(
                            out=vaug[:, VW * i + 64: VW * i + 66], in_=ones2)
                        nc.vector.tensor_copy(
                            out=vaug[:, VW * i + 130: VW * i + 132], in_=ones2)

            # ---- stage C: flash attention + interleaved output projection ----
            with tc.tile_pool(name="stp", bufs=2, space="PSUM") as stp, \
                 tc.tile_pool(name="ctxp", bufs=1, space="PSUM") as ctxp, \
                 tc.tile_pool(name="psD", bufs=1, space="PSUM") as psDp, \
                 tc.tile_pool(name="ptp", bufs=6) as ptp, \
                 tc.tile_pool(name="drp", bufs=2, space="DRAM") as drp, \
                 tc.tile_pool(name="sC", bufs=2) as sC, \
                 tc.tile_pool(name="sD", bufs=3) as sD:

                saved = {}   # qb -> (ctx_sb, rl)

                def stage_d_chunk(qb, c):
                    ctx_sb, rl = saved[qb]
                    ps_a = psDp.tile([128, 512], F32, tag="psoA",
                                     name=f"psA_{qb}_{c}")
                    nc.tensor.matmul(
                        ps_a, ctx_sb[0:64, 128 * c: 128 * (c + 1)],
                        woTs[0:64, :], start=True, stop=True)
                    ps_b = psDp.tile([128, 512], F32, tag="psoB",
                                     name=f"psB_{qb}_{c}")
                    nc.tensor.matmul(
                        ps_b, ctx_sb[64:128, 128 * c: 128 * (c + 1)],
                        woTs[64:128, :], start=True, stop=True)
                    tmp = sD.tile([128, 512], F32, tag="tmp",
                                  name=f"tmp_{qb}_{c}")
                    nc.vector.tensor_scalar_mul(
                        out=tmp, in0=ps_b, scalar1=rl[:, 1, c:c + 1])
                    ost = sD.tile([128, 512], F32, tag="ost",
                                  name=f"ost_{qb}_{c}")
                    nc.vector.scalar_tensor_tensor(
                        out=ost, in0=ps_a, scalar=rl[:, 0, c:c + 1], in1=tmp,
                        op0=mybir.AluOpType.mult, op1=mybir.AluOpType.add)
                    r0 = QB * qb + 128 * c
                    nc.sync.dma_start(out=part[r0: r0 + 128, :], in_=ost)

                for qb in range(NQB):
                    qsl = slice(QB * qb, QB * (qb + 1))
                    ps_c0 = ctxp.tile([66, 512], F32, tag="c0",
                                      name=f"psc0_{qb}")
                    ps_c1 = ctxp.tile([66, 512], F32, tag="c1",
                                      name=f"psc1_{qb}")

                    def pv_pair(kb, pt):
                        nc.tensor.matmul(
                            ps_c0, vaug[:, VW * kb: VW * kb + 66],
                            pt[:, 0:512],
                            start=(kb == 0), stop=(kb == TT - 1))
                        nc.tensor.matmul(
                            ps_c1, vaug[:, VW * kb + 66: VW * kb + 132],
                            pt[:, 512:1024],
                            start=(kb == 0), stop=(kb == TT - 1))

                    pv_pending = None
                    for kb in range(TT):
                        st = stp.tile([128, 1024], F32, tag="st",
                                      name=f"st_{qb}_{kb}")
                        nc.tensor.matmul(
                            st[:, 0:512],
                            kT2[0:64, 128 * kb: 128 * (kb + 1)],
                            qT2[0:64, qsl], start=True, stop=True)
                        nc.tensor.matmul(
                            st[:, 512:1024],
                            kT2[64:128, 128 * kb: 128 * (kb + 1)],
                            qT2[64:128, qsl], start=True, stop=True)
                        pt = ptp.tile([128, 1024], BF16, tag="pt",
                                      name=f"pt_{qb}_{kb}")
                        if kb % DVE_EXP_MOD == DVE_EXP_MOD - 1:
                            nc.vector.tensor_scalar(
                                out=pt.bitcast(I16), in0=st,
                                scalar1=EXPA, scalar2=EXPB,
                                op0=mybir.AluOpType.mult,
                                op1=mybir.AluOpType.add)
                        else:
                            nc.scalar.activation(
                                out=pt, in_=st,
                                func=mybir.ActivationFunctionType.Exp)
                        if qb > 0 and 1 <= kb <= 4:
                            stage_d_chunk(qb - 1, kb - 1)
                        if pv_pending is not None:
                            pv_pair(*pv_pending)
                        pv_pending = (kb, pt)
                    pv_pair(*pv_pending)
                    pv_pending = None
                    # ---- qblock tail: extract ctx (bf16) and 1/l ----
                    ctx_sb = sC.tile([128, QB], BF16, tag="ctx",
                                     name=f"ctx_{qb}")
                    nc.vector.tensor_copy(out=ctx_sb[0:64, :], in_=ps_c0[0:64, :])
                    nc.vector.tensor_copy(out=ctx_sb[64:128, :], in_=ps_c1[0:64, :])
                    l_sb = sC.tile([1, 2 * QB], F32, tag="lsb", name=f"lsb_{qb}")
                    nc.vector.tensor_copy(out=l_sb[0:1, 0:QB], in_=ps_c0[64:65, :])
                    nc.vector.tensor_copy(out=l_sb[0:1, QB:2 * QB],
                                          in_=ps_c1[64:65, :])
                    ld = drp.tile([1, 2 * QB], F32, tag="ld", name=f"ld_{qb}")
                    nc.sync.dma_start(out=ld, in_=l_sb)
                    # transposed read-back: rl_t[p, h, i] = l[0, 512*h + 128*i + p]
                    rl_t = sC.tile([128, 2, 4], F32, tag="rlt",
                                   name=f"rlt_{qb}")
                    nc.gpsimd.dma_start(
                        out=rl_t, in_=_ap(ld, [[1, 128], [QB, 2], [128, 4]]))
                    rl = sC.tile([128, 2, 4], F32, tag="rl", name=f"rl_{qb}")
                    nc.vector.reciprocal(rl, rl_t)
                    saved[qb] = (ctx_sb, rl)
                for c in range(4):
                    stage_d_chunk(NQB - 1, c)

    split_excess_waits(nc)
    return nc


_NC_CACHE = None


def _get_nc():
    global _NC_CACHE
    if _NC_CACHE is None:
        _NC_CACHE = build_kernel()
    return _NC_CACHE


def make_in_maps(x, Wq, bq, Wk, bk, Wv, bv, Wo, bo):
    scale = 1.0 / np.sqrt(DK)
    bf = ml_dtypes.bfloat16
    in_maps = []
    for core in range(N_CORES):
        b, hp = divmod(core, 4)
        R = slice(128 * hp, 128 * hp + 128)
        in_maps.append({
            "xbT": np.ascontiguousarray(x[b].T.astype(bf)),
            "wqT": np.ascontiguousarray((Wq[R] * scale).T.astype(bf)),
            "wkT": np.ascontiguousarray(Wk[R].T.astype(bf)),
            "wvT": np.ascontiguousarray(Wv[R].T.astype(bf)),
            "woT": np.ascontiguousarray(Wo[:, R].T, dtype=np.float32),
            "bq": np.ascontiguousarray(
                (bq[R] * scale).reshape(128, 1), dtype=np.float32),
        })
    return in_maps


def kernel(x, Wq, bq, Wk, bk, Wv, bv, Wo, bo):
    x = np.asarray(x, dtype=np.float32)
    Wq, Wk, Wv, Wo = (np.asarray(a, dtype=np.float32) for a in (Wq, Wk, Wv, Wo))
    bq, bk, bv, bo = (np.asarray(a, dtype=np.float32) for a in (bq, bk, bv, bo))

    nc = _get_nc()
    in_maps = make_in_maps(x, Wq, bq, Wk, bk, Wv, bv, Wo, bo)
    res = run_bass_kernel_spmd(nc, in_maps, list(range(N_CORES)))
    parts = [res.results[c]["part"] for c in range(N_CORES)]

    # bk only shifts every score in a q column equally -> softmax-invariant,
    # so it is dropped on device. bv/bo contributions fold out exactly too.
    bcorr = (bv @ Wo.T + bo).astype(np.float32)
    out = np.empty((B, T, D), dtype=np.float32)
    for b in range(B):
        acc = parts[4 * b].astype(np.float64)
        for c in range(4 * b + 1, 4 * b + 4):
            acc += parts[c]
        out[b] = (acc + bcorr).astype(np.float32)
    return out
